# revision 31
# baseline (speedup 1.0000x reference)
"""Trainium2 Bass kernel for GQA attention (B=4, S=2048, H=576, 9 heads / 3 KV groups, RoPE).

Sharding: 8 cores = (batch b, seq-half) pairs. Each core computes the full
attention output for 1024 query rows of one batch element (keys/values over
the full 2048 positions of that batch element are recomputed locally).

Schedule: attention (the ACT-engine exp stream, which is the bottleneck)
starts as soon as K groups 0/1 (first seq half) + Q chunk 0 are projected;
all remaining projections (V chunks, Q chunks 1-4, K second half / group 2)
are drip-fed into the attention pair loop as small "extras" that fill
TensorE slack while ACT runs.

RoPE is computed as q' = q*cos + (P.T @ q)*sin where P is a constant 128x128
block permutation carrying the rotate-half signs (one small matmul replaces
the per-32-row DVE cross-multiplies).

Layout: features on partitions, seq on free dim (all "transposed"):
  QT = wq @ hsT, KT = wk @ hsT, V natural [s, hv]
  ST[k, q] = K-chunk.T @ QT -> exp on ACT -> attnT fp16
  av[hd+1, q] = [V | ones].T @ attnT  (ones row = softmax denominator)
  outT = woT.T @ (av[0:64] / av[64])
Head pairs (2h, 2h+1) run concurrently on the PE array via 64x128 row tiling
(lhsT/rhs in partitions 0-63 vs 64-127). Matmul inputs fp16, fp32 PSUM.
"""

import sys

if "/opt/trn_rl_repo" not in sys.path:
    sys.path.insert(0, "/opt/trn_rl_repo")

import numpy as np

import concourse.bass as bass
import concourse.mybir as mybir
import concourse.tile as tile
from concourse import bacc
from concourse.bass_utils import run_bass_kernel_spmd

F16 = mybir.dt.float16
F32 = mybir.dt.float32

B = 4
S = 2048
SQ = 1024  # query rows per core
H = 576
NH = 9
HD = 64
KV = 192
G = 3
ROPE_THETA = 10000.0
SCALE = 1.0 / 8.0  # 1/sqrt(HD)
# Schraudolph exp2-bit-trick constants: exp(SCALE*x) = 2^(SCALE*log2(e)*x)
EXP2_A1 = SCALE * 1.4426950408889634 * 1024.0
EXP2_A2 = 15360.0 - 44.0

NDC = 5  # contraction chunks over hidden (4*128 + 64)
NEC = 5  # output feature chunks (4*128 + 64)
NKC = S // 128  # 16 key chunks
PAIRS = [(0, 1), (2, 3), (4, 5), (6, 7), (8,)]


def _dcm(dc):
    return min(128, H - dc * 128)  # 128,128,128,128,64


def _host_tables():
    inv_freq = 1.0 / (ROPE_THETA ** (np.arange(0, HD, 2, dtype=np.float32) / HD))
    t = np.arange(S, dtype=np.float32)
    freqs = np.einsum("i,j->ij", inv_freq, t)  # [32, S]
    cos32 = np.cos(freqs).astype(np.float16)
    sin32 = np.sin(freqs).astype(np.float16)
    # P[c, r]: rot = P.T @ x; rot[r] = -x[r+32] (r in [0,32)), +x[r-32] (r in [32,64))
    prot = np.zeros((128, 128), np.float16)
    for b0 in (0, 64):
        for j in range(32):
            prot[b0 + j + 32, b0 + j] = -1.0
            prot[b0 + j, b0 + j + 32] = 1.0
    return cos32, sin32, prot


def _build_bass():
    nc = bacc.Bacc("TRN2", target_bir_lowering=False)

    hsT = nc.declare_dram_parameter("hsT", [H, S], F16, isOutput=False)
    wqkvT = nc.declare_dram_parameter("wqkvT", [H, 960], F16, isOutput=False)
    woT = nc.declare_dram_parameter("woT", [H, H], F16, isOutput=False)
    trig = nc.declare_dram_parameter("trig", [32, 2 * S], F16, isOutput=False)
    prot = nc.declare_dram_parameter("prot", [128, 128], F16, isOutput=False)
    out = nc.declare_dram_parameter("o", [H, SQ], F16, isOutput=True)

    with tile.TileContext(nc) as tc:
        kernel_body(nc, tc, hsT, wqkvT, woT, trig, prot, out)

    nc.compile()
    return nc


def kernel_body(nc, tc, hsT, wqkvT, woT, trig, prot, out):
    import contextlib
    from collections import deque

    ctx = contextlib.ExitStack()
    with ctx:
        wpool = ctx.enter_context(tc.tile_pool(name="w", bufs=1))
        qtp = ctx.enter_context(tc.tile_pool(name="qt", bufs=1))
        ktp = ctx.enter_context(tc.tile_pool(name="kt", bufs=1))
        vap = ctx.enter_context(tc.tile_pool(name="va", bufs=1))
        otp = ctx.enter_context(tc.tile_pool(name="ot", bufs=1))
        ropep = ctx.enter_context(tc.tile_pool(name="rope", bufs=2))
        attnp = ctx.enter_context(tc.tile_pool(name="attn", bufs=6))
        miscp = ctx.enter_context(tc.tile_pool(name="misc", bufs=3))
        ps = ctx.enter_context(tc.tile_pool(name="ps", bufs=1, space="PSUM"))

        # ---------------- inputs to SBUF ----------------
        # sync queue: only small latency-critical transfers (trig, prot, the
        # ktd/qt row-duplications later, output).  Bulk loads go on the
        # gpsimd and scalar queues, split per (dc, seq-half) so the first
        # K/Q projection matmuls can start as soon as their chunk lands.
        hs_big = wpool.tile([128, NDC, S], F16, tag="hsb", name="hs_big")
        wqkv_sb = wpool.tile([128, NDC, 960], F16, tag="wqkv", name="wqkv_sb")
        wo_sb = wpool.tile([128, NEC, H], F16, tag="wo", name="wo_sb")
        trig_sb = wpool.tile([128, 2 * S], F16, tag="trig", name="trig_sb")
        prot_sb = wpool.tile([128, 128], F16, tag="prot", name="prot_sb")

        # sync (Q1): trig/prot then first-half hs chunks 0-2, later dups.
        # gpsimd (Q0): all wqkv chunks.  scalar (Q10): trig replication,
        # first-half hs chunks 3-4, then second-half hs, then wo.
        nc.sync.dma_start(out=trig_sb[0:32, :], in_=trig[:, :])
        nc.sync.dma_start(out=prot_sb, in_=prot[:, :])
        # prefix-critical loads (wqkv + first-half hs) balanced across all
        # three queues, interleaved by dc so early chunks land first
        qs = [nc.sync, nc.gpsimd, nc.scalar]
        qi = 0
        for dc in range(NDC):
            m = _dcm(dc)
            r0 = dc * 128
            qs[qi % 3].dma_start(out=wqkv_sb[:m, dc, :], in_=wqkvT[r0 : r0 + m, :])
            qi += 1
            qs[qi % 3].dma_start(out=hs_big[:m, dc, 0:SQ], in_=hsT[r0 : r0 + m, 0:SQ])
            qi += 1
        for r in range(1, 4):
            nc.scalar.dma_start(
                out=trig_sb[r * 32 : (r + 1) * 32, :], in_=trig_sb[0:32, :]
            )
        for dc in range(NDC):
            m = _dcm(dc)
            r0 = dc * 128
            qs[dc % 2].dma_start(out=hs_big[:m, dc, SQ:S], in_=hsT[r0 : r0 + m, SQ:S])
        nc.scalar.dma_start(
            out=wo_sb[:, 0:4, :], in_=woT[0:512, :].rearrange("(n p) c -> p n c", p=128)
        )
        nc.scalar.dma_start(out=wo_sb[:64, 4, :], in_=woT[512:576, :])

        cos_ap = trig_sb[:, 0:S]
        sin_ap = trig_sb[:, S : 2 * S]

        # persistent activation tensors
        qt_sb = [qtp.tile([128, SQ], F16, tag=f"qt{c}", name=f"qt{c}") for c in range(NEC)]
        ktd_sb = [ktp.tile([128, S], F16, tag=f"ktd{g}", name=f"ktd{g}") for g in range(G)]
        va_sb = [vap.tile([128, 3 * 65], F16, tag=f"va{kc}", name=f"va{kc}") for kc in range(NKC)]
        ot_sb = [otp.tile([128, SQ], F16, tag=f"ot{c}", name=f"ot{c}") for c in range(NEC)]

        # ---------------- rope (split into a/b halves for pipelining) ----
        def rope_a(src_ps, n, so):
            """Evacuate PSUM proj -> fp16 raw, and compute raw*cos."""
            raw = ropep.tile([128, SQ], F16, tag="rraw", name="raw")
            nc.vector.tensor_copy(raw[:n, :], src_ps[:n, :])
            tcm = ropep.tile([128, SQ], F16, tag="rtcm", name="tcm")
            nc.vector.tensor_mul(tcm[:n, :], raw[:n, :], cos_ap[:n, so : so + SQ])
            return raw, tcm

        def rope_b(raw, tcm, n, so, dst_writes):
            """rot = P.T @ raw (PE); dst = tcm + rot*sin."""
            rot = ps.tile([128, SQ], F32, tag="pj", name="rot")
            for sb in range(2):
                nc.tensor.matmul(
                    rot[:n, sb * 512 : (sb + 1) * 512],
                    lhsT=prot_sb[:n, :n],
                    rhs=raw[:n, sb * 512 : (sb + 1) * 512],
                    start=True,
                    stop=True,
                )
            tsm = ropep.tile([128, SQ], F16, tag="rtsm", name="tsm")
            nc.vector.tensor_mul(tsm[:n, :], rot[:n, :], sin_ap[:n, so : so + SQ])
            for dst, row, r in dst_writes:
                nc.vector.tensor_add(dst, tcm[row : row + r, :], tsm[row : row + r, :])

        # ---------------- projections ----------------
        def k_proj_mm(piece, ch, part):
            # part: 0 -> dc 0-2, 1 -> dc 3-4, None -> all (dc-major so the
            # stationary weight load is shared between the two seq halves)
            roff, m = (0, 128) if ch == 0 else (128, 64)
            so = piece * SQ
            if part != 1:
                kps = ps.tile([128, SQ], F32, tag="pj", name=f"kps{piece}{ch}")
                k_proj_mm.cur = kps
            else:
                kps = k_proj_mm.cur
            dcs = {0: range(0, 3), 1: range(3, NDC), None: range(NDC)}[part]
            for dc in dcs:
                p = _dcm(dc)
                for s2 in range(2):
                    nc.tensor.matmul(
                        kps[:m, s2 * 512 : (s2 + 1) * 512],
                        lhsT=wqkv_sb[:p, dc, 576 + roff : 576 + roff + m],
                        rhs=hs_big[:p, dc, so + s2 * 512 : so + (s2 + 1) * 512],
                        start=(dc == 0),
                        stop=(dc == NDC - 1),
                    )
            return kps

        def k_rope(piece, ch, kps):
            m = 128 if ch == 0 else 64
            so = piece * SQ
            raw, tcm = rope_a(kps, m, so)
            if ch == 0:
                writes = [
                    (ktd_sb[0][0:64, so : so + SQ], 0, 64),
                    (ktd_sb[1][0:64, so : so + SQ], 64, 64),
                ]
                groups = (0, 1)
            else:
                writes = [(ktd_sb[2][0:64, so : so + SQ], 0, 64)]
                groups = (2,)
            rope_b(raw, tcm, m, so, writes)
            for g in groups:
                nc.sync.dma_start(
                    out=ktd_sb[g][64:128, so : so + SQ],
                    in_=ktd_sb[g][0:64, so : so + SQ],
                )

        def q_proj_mm(c, part):
            m = min(128, H - c * 128)
            if part != 1:
                qps = ps.tile([128, SQ], F32, tag="pj", name=f"qps{c}")
                q_proj_mm.cur = qps
            else:
                qps = q_proj_mm.cur
            dcs = {0: range(0, 3), 1: range(3, NDC), None: range(NDC)}[part]
            for dc in dcs:
                p = _dcm(dc)
                for s2 in range(2):
                    nc.tensor.matmul(
                        qps[:m, s2 * 512 : (s2 + 1) * 512],
                        lhsT=wqkv_sb[:p, dc, c * 128 : c * 128 + m],
                        rhs=hs_big[:p, dc, s2 * 512 : (s2 + 1) * 512],
                        start=(dc == 0),
                        stop=(dc == NDC - 1),
                    )
            return qps

        def q_rope(c, qps):
            m = min(128, H - c * 128)
            raw, tcm = rope_a(qps, m, 0)
            rope_b(raw, tcm, m, 0, [(qt_sb[c][0:m, :], 0, m)])
            if c == 4:
                # duplicate head-8 rows into upper partitions for the row-tiled
                # qb=1 score matmul of the final (single-head) pair
                nc.sync.dma_start(
                    out=qt_sb[4][64:128, 512:1024], in_=qt_sb[4][0:64, 512:1024]
                )

        def v_proj(kc):
            vps = ps.tile([128, SQ], F32, tag="pj", name=f"vps{kc}")
            for dc in range(NDC):
                p = _dcm(dc)
                nc.tensor.matmul(
                    vps[:, :KV],
                    lhsT=hs_big[:p, dc, kc * 128 : (kc + 1) * 128],
                    rhs=wqkv_sb[:p, dc, 768:960],
                    start=(dc == 0),
                    stop=(dc == NDC - 1),
                )
            # only the 3 "ones" columns need the memset; V overwrites the rest
            nc.vector.memset(
                va_sb[kc].rearrange("p (g w) -> p g w", g=G)[:, :, 64:65], 1.0
            )
            dst = va_sb[kc].rearrange("p (g w) -> p g w", g=G)[:, :, 0:64]
            srcv = vps[:, :KV].rearrange("p (g w) -> p g w", g=G)
            nc.vector.tensor_copy(dst, srcv)

        # ---------------- PE warm-up ----------------
        # The HAM clock gate holds the PE at 1.2 GHz until it has been busy
        # ~3.4us.  Issue dependency-free dummy matmuls on scratch SBUF so the
        # array is at 2.4 GHz by the time the first real projection data
        # lands (~12-14us in, DMA-bound).
        wu_s = wpool.tile([128, 512], F16, tag="wus", name="wu_s")
        nc.vector.memset(wu_s, 0.125)
        wu_ps = ps.tile([128, SQ], F32, tag="pj", name="wu_ps")
        for _ in range(40):
            nc.tensor.matmul(
                wu_ps[:, 0:512], lhsT=wu_s[:, 0:128], rhs=wu_s, start=True, stop=True
            )

        # ---------------- prefix: work before attention ----------
        # K both pieces (groups 0/1), Q chunk 0, V chunks 0-1.  Runs warm
        # thanks to the dummies above, overlapping the remaining DMA.
        kps = k_proj_mm(0, 0, None)
        k_rope(0, 0, kps)
        qps = q_proj_mm(0, None)
        q_rope(0, qps)
        for kc in range(4):
            v_proj(kc)

        # ---------------- extras: deferred projections -------------------
        extras = deque()

        def add_proj_extras(fn_mm, fn_rope, *args, spread=0):
            st = {}

            def e1():
                st["ps"] = fn_mm(*args, 0)

            def e2():
                fn_mm(*args, 1)

            def e3():
                fn_rope(*args, st["ps"])

            for e in (e1, e2, e3):
                extras.append(e)
                for _ in range(spread):
                    extras.append(None)

        add_proj_extras(k_proj_mm, k_rope, 1, 0)  # K piece 1 (needed at kc=8)
        for kc in range(4, NKC):
            extras.append(lambda kc=kc: v_proj(kc))
        add_proj_extras(q_proj_mm, q_rope, 1, spread=1)
        add_proj_extras(q_proj_mm, q_rope, 2, spread=1)
        add_proj_extras(q_proj_mm, q_rope, 3, spread=1)
        add_proj_extras(k_proj_mm, k_rope, 0, 1, spread=1)
        add_proj_extras(k_proj_mm, k_rope, 1, 1, spread=1)
        add_proj_extras(q_proj_mm, q_rope, 4, spread=1)

        # ---------------- attention ----------------
        def attn_pair(pi):
            pair = PAIRS[pi]
            two = len(pair) == 2
            hA = pair[0]
            gA = hA // 3
            gB = pair[1] // 3 if two else gA
            c = hA // 2
            for qb in (0, 1) if two else (0,):
                avA = ps.tile([65, 512], F32, tag="avA", bufs=1, name="avA")
                avB = ps.tile([65, 512], F32, tag="avB", bufs=1, name="avB")
                at_tiles = [None] * NKC

                def emit_av(kcav):
                    nc.tensor.matmul(
                        avA,
                        lhsT=va_sb[kcav][:, gA * 65 : gA * 65 + 65],
                        rhs=at_tiles[kcav][:, 0:512],
                        start=(kcav == 0),
                        stop=(kcav == NKC - 1),
                    )
                    nc.tensor.matmul(
                        avB,
                        lhsT=va_sb[kcav][:, gB * 65 : gB * 65 + 65],
                        rhs=at_tiles[kcav][:, 512:1024],
                        start=(kcav == 0),
                        stop=(kcav == NKC - 1),
                    )

                for kc in range(NKC):
                    st = ps.tile([128, 1024], F32, tag="st", bufs=2, name="st")
                    if two:
                        rhsA = qt_sb[c][0:64, qb * 512 : (qb + 1) * 512]
                        rhsB = qt_sb[c][64:128, qb * 512 : (qb + 1) * 512]
                    else:
                        rhsA = qt_sb[c][0:64, 0:512]
                        rhsB = qt_sb[c][64:128, 512:1024]
                    nc.tensor.matmul(
                        st[:, 0:512],
                        lhsT=ktd_sb[gA][0:64, kc * 128 : (kc + 1) * 128],
                        rhs=rhsA,
                        start=True,
                        stop=True,
                    )
                    nc.tensor.matmul(
                        st[:, 512:1024],
                        lhsT=ktd_sb[gB][64:128, kc * 128 : (kc + 1) * 128],
                        rhs=rhsB,
                        start=True,
                        stop=True,
                    )
                    at_t = attnp.tile([128, 1024], F16, tag="at", name="at")
                    if kc in (2, 5, 8, 11):
                        # offload this step's exp to the vector engine via the
                        # Schraudolph exp2 bit trick: int16 bits = u*1024 +
                        # (15360 - 44) with u = st*scale*log2(e), reinterpreted
                        # as fp16 ~= 2^u (max rel err ~3%, zero-mean across
                        # keys; numerator/denominator use the same weights so
                        # the bias cancels).
                        nc.vector.tensor_scalar(
                            at_t.bitcast(mybir.dt.int16),
                            st,
                            EXP2_A1,
                            EXP2_A2,
                            mybir.AluOpType.mult,
                            mybir.AluOpType.add,
                        )
                    else:
                        nc.scalar.activation(
                            at_t, st, mybir.ActivationFunctionType.Exp, scale=SCALE
                        )
                    at_tiles[kc] = at_t
                    if extras:
                        e = extras.popleft()
                        if e is not None:
                            e()
                    if kc > 0:
                        emit_av(kc - 1)

                emit_av(NKC - 1)

                targets = (
                    [(hA, avA, qb), (pair[1], avB, qb)]
                    if two
                    else [(hA, avA, 0), (hA, avB, 1)]
                )
                # Evacuate both av PSUM banks first (so the next pair's AV
                # matmuls can reuse them ASAP), then do the arithmetic on the
                # SBUF copies.  The denominator row must be staged to a
                # partition-0 tile (custom DVE ops drop partition offsets).
                stage = []
                for h, av, qbx in targets:
                    avs = miscp.tile([64, 512], F32, tag="avs", name="avs")
                    nc.vector.tensor_copy(avs, av[0:64, :])
                    dn = miscp.tile([1, 512], F32, tag="dn", name="dn")
                    nc.vector.tensor_copy(dn, av[64:65, :])
                    stage.append((h, qbx, avs, dn))
                rds = []
                for h, qbx, avs, dn in stage:
                    rd = miscp.tile([1, 512], F32, tag="rd", name="rd")
                    nc.vector.reciprocal_approx_fast(out=rd, in_=dn)
                    rds.append(rd)
                for (h, qbx, avs, dn), rd in zip(stage, rds):
                    bc = miscp.tile([64, 512], F32, tag="bc", name="bc")
                    nc.gpsimd.partition_broadcast(bc, rd)
                    row = (h % 2) * 64
                    # final scale on gpsimd (SBUF-only operands) to keep the
                    # DVE free for the offloaded exp work
                    nc.gpsimd.tensor_mul(
                        ot_sb[h // 2][row : row + 64, qbx * 512 : (qbx + 1) * 512],
                        avs,
                        bc,
                    )

        for pi in range(len(PAIRS)):
            attn_pair(pi)

        # ---------------- output projection ----------------
        # Keep the PE busy through the final normalize (DVE/gpsimd) so HAM
        # does not re-throttle right before the o_proj matmuls.
        wu2 = ps.tile([128, SQ], F32, tag="pj", name="wu2")
        for _ in range(10):
            nc.tensor.matmul(
                wu2[:, 0:512], lhsT=wu_s[:, 0:128], rhs=wu_s, start=True, stop=True
            )
        # attention is done: the "st" PSUM buffers (2x) are free, giving a
        # double-buffered ec pipeline; fp16 output halves the writeback.
        dma_engs = [nc.sync, nc.gpsimd, nc.scalar]
        for ec in range(NEC):
            m = min(128, H - ec * 128)
            ft = ps.tile([128, SQ], F32, tag="st", bufs=2, name=f"ft{ec}")
            for sb in range(2):
                for cc in range(NEC):
                    k = _dcm(cc)
                    nc.tensor.matmul(
                        ft[:m, sb * 512 : (sb + 1) * 512],
                        lhsT=wo_sb[:k, cc, ec * 128 : ec * 128 + m],
                        rhs=ot_sb[cc][:k, sb * 512 : (sb + 1) * 512],
                        start=(cc == 0),
                        stop=(cc == NEC - 1),
                    )
                fts = miscp.tile([128, 512], F16, tag="fts", name="fts")
                if sb == 0:
                    nc.scalar.copy(fts[:m, :], ft[:m, 0:512])
                else:
                    nc.vector.tensor_copy(fts[:m, :], ft[:m, 512:1024])
                dma_engs[(ec * 2 + sb) % 3].dma_start(
                    out=out[ec * 128 : ec * 128 + m, sb * 512 : (sb + 1) * 512],
                    in_=fts[:m, :],
                )


_NC_CACHE = {}


def _get_nc():
    if "nc" not in _NC_CACHE:
        _NC_CACHE["nc"] = _build_bass()
    return _NC_CACHE["nc"]


def kernel(hidden_states, wq, wk, wv, wo):
    cos32, sin32, prot = _host_tables()

    wqkv = np.empty((H, 960), np.float16)
    wqkv[:, 0:H] = wq.T.astype(np.float16)
    wqkv[:, H : H + KV] = wk.T.astype(np.float16)
    wqkv[:, H + KV : H + 2 * KV] = wv.T.astype(np.float16)
    wo16 = wo.T.astype(np.float16)

    trig0 = np.concatenate([cos32, sin32], axis=1)
    trig1 = np.concatenate(
        [np.roll(cos32, -SQ, axis=1), np.roll(sin32, -SQ, axis=1)], axis=1
    )

    in_maps = []
    core_ids = list(range(8))
    for c in core_ids:
        b, half = c // 2, c % 2
        hsT16 = hidden_states[b].T.astype(np.float16)
        if half == 1:
            # roll so this core's queries sit at columns [0, SQ); keys keep
            # their correct rope position via the equally-rolled cos/sin.
            hsT16 = np.roll(hsT16, -SQ, axis=1)
        in_maps.append(
            {
                "hsT": hsT16,
                "wqkvT": wqkv,
                "woT": wo16,
                "trig": trig0 if half == 0 else trig1,
                "prot": prot,
            }
        )

    global _LAST_IN_MAPS
    _LAST_IN_MAPS = in_maps
    nc = _get_nc()
    res = run_bass_kernel_spmd(nc, in_maps, core_ids=core_ids)

    out = np.empty((B, S, H), np.float32)
    for c in core_ids:
        b, half = c // 2, c % 2
        out[b, half * SQ : (half + 1) * SQ, :] = res.results[c]["o"].T.astype(
            np.float32
        )
    return out


if __name__ == "__main__":
    rng = np.random.default_rng(0)
    hs = rng.standard_normal((B, S, H), dtype=np.float32)
    s = 1.0 / np.sqrt(H)
    wq = rng.standard_normal((H, H), dtype=np.float32) * s
    wk = rng.standard_normal((KV, H), dtype=np.float32) * s
    wv = rng.standard_normal((KV, H), dtype=np.float32) * s
    wo = rng.standard_normal((H, H), dtype=np.float32) * s
    o = kernel(hidden_states=hs, wq=wq, wk=wk, wv=wv, wo=wo)
    print(o.shape, o.dtype, np.abs(o).mean())


# revision 33
# speedup vs baseline: 1.0209x; 1.0209x over previous
"""Trainium2 Bass kernel for GQA attention (B=4, S=2048, H=576, 9 heads / 3 KV groups, RoPE).

Sharding: 8 cores = (batch b, seq-half) pairs. Each core computes the full
attention output for 1024 query rows of one batch element (keys/values over
the full 2048 positions of that batch element are recomputed locally).

Schedule: attention (the ACT-engine exp stream, which is the bottleneck)
starts as soon as K groups 0/1 (first seq half) + Q chunk 0 are projected;
all remaining projections (V chunks, Q chunks 1-4, K second half / group 2)
are drip-fed into the attention pair loop as small "extras" that fill
TensorE slack while ACT runs.

RoPE is computed as q' = q*cos + (P.T @ q)*sin where P is a constant 128x128
block permutation carrying the rotate-half signs (one small matmul replaces
the per-32-row DVE cross-multiplies).

Layout: features on partitions, seq on free dim (all "transposed"):
  QT = wq @ hsT, KT = wk @ hsT, V natural [s, hv]
  ST[k, q] = K-chunk.T @ QT -> exp on ACT -> attnT fp16
  av[hd+1, q] = [V | ones].T @ attnT  (ones row = softmax denominator)
  outT = woT.T @ (av[0:64] / av[64])
Head pairs (2h, 2h+1) run concurrently on the PE array via 64x128 row tiling
(lhsT/rhs in partitions 0-63 vs 64-127). Matmul inputs fp16, fp32 PSUM.
"""

import sys

if "/opt/trn_rl_repo" not in sys.path:
    sys.path.insert(0, "/opt/trn_rl_repo")

import numpy as np

import concourse.bass as bass
import concourse.mybir as mybir
import concourse.tile as tile
from concourse import bacc
from concourse.bass_utils import run_bass_kernel_spmd

F16 = mybir.dt.float16
F32 = mybir.dt.float32

B = 4
S = 2048
SQ = 1024  # query rows per core
H = 576
NH = 9
HD = 64
KV = 192
G = 3
ROPE_THETA = 10000.0
SCALE = 1.0 / 8.0  # 1/sqrt(HD)
# Schraudolph exp2-bit-trick constants: exp(SCALE*x) = 2^(SCALE*log2(e)*x)
EXP2_A1 = SCALE * 1.4426950408889634 * 1024.0
EXP2_A2 = 15360.0 - 44.0

NDC = 5  # contraction chunks over hidden (4*128 + 64)
NEC = 5  # output feature chunks (4*128 + 64)
NKC = S // 128  # 16 key chunks
PAIRS = [(0, 1), (2, 3), (4, 5), (6, 7), (8,)]


def _dcm(dc):
    return min(128, H - dc * 128)  # 128,128,128,128,64


def _host_tables():
    inv_freq = 1.0 / (ROPE_THETA ** (np.arange(0, HD, 2, dtype=np.float32) / HD))
    t = np.arange(S, dtype=np.float32)
    freqs = np.einsum("i,j->ij", inv_freq, t)  # [32, S]
    cos32 = np.cos(freqs).astype(np.float16)
    sin32 = np.sin(freqs).astype(np.float16)
    # P[c, r]: rot = P.T @ x; rot[r] = -x[r+32] (r in [0,32)), +x[r-32] (r in [32,64))
    prot = np.zeros((128, 128), np.float16)
    for b0 in (0, 64):
        for j in range(32):
            prot[b0 + j + 32, b0 + j] = -1.0
            prot[b0 + j, b0 + j + 32] = 1.0
    return cos32, sin32, prot


def _build_bass():
    nc = bacc.Bacc("TRN2", target_bir_lowering=False)

    hsT = nc.declare_dram_parameter("hsT", [H, S], F16, isOutput=False)
    wqkvT = nc.declare_dram_parameter("wqkvT", [H, 960], F16, isOutput=False)
    woT = nc.declare_dram_parameter("woT", [H, H], F16, isOutput=False)
    trig = nc.declare_dram_parameter("trig", [32, 2 * S], F16, isOutput=False)
    prot = nc.declare_dram_parameter("prot", [128, 128], F16, isOutput=False)
    out = nc.declare_dram_parameter("o", [H, SQ], F16, isOutput=True)

    with tile.TileContext(nc) as tc:
        kernel_body(nc, tc, hsT, wqkvT, woT, trig, prot, out)

    nc.compile()
    return nc


def kernel_body(nc, tc, hsT, wqkvT, woT, trig, prot, out):
    import contextlib
    from collections import deque

    ctx = contextlib.ExitStack()
    with ctx:
        wpool = ctx.enter_context(tc.tile_pool(name="w", bufs=1))
        qtp = ctx.enter_context(tc.tile_pool(name="qt", bufs=1))
        ktp = ctx.enter_context(tc.tile_pool(name="kt", bufs=1))
        vap = ctx.enter_context(tc.tile_pool(name="va", bufs=1))
        otp = ctx.enter_context(tc.tile_pool(name="ot", bufs=1))
        ropep = ctx.enter_context(tc.tile_pool(name="rope", bufs=2))
        attnp = ctx.enter_context(tc.tile_pool(name="attn", bufs=6))
        miscp = ctx.enter_context(tc.tile_pool(name="misc", bufs=3))
        ps = ctx.enter_context(tc.tile_pool(name="ps", bufs=1, space="PSUM"))

        # ---------------- inputs to SBUF ----------------
        # sync queue: only small latency-critical transfers (trig, prot, the
        # ktd/qt row-duplications later, output).  Bulk loads go on the
        # gpsimd and scalar queues, split per (dc, seq-half) so the first
        # K/Q projection matmuls can start as soon as their chunk lands.
        hs_big = wpool.tile([128, NDC, S], F16, tag="hsb", name="hs_big")
        wqkv_sb = wpool.tile([128, NDC, 960], F16, tag="wqkv", name="wqkv_sb")
        wo_sb = wpool.tile([128, NEC, H], F16, tag="wo", name="wo_sb")
        trig_sb = wpool.tile([128, 2 * S], F16, tag="trig", name="trig_sb")
        prot_sb = wpool.tile([128, 128], F16, tag="prot", name="prot_sb")

        # sync (Q1): trig/prot then first-half hs chunks 0-2, later dups.
        # gpsimd (Q0): all wqkv chunks.  scalar (Q10): trig replication,
        # first-half hs chunks 3-4, then second-half hs, then wo.
        nc.sync.dma_start(out=trig_sb[0:32, :], in_=trig[:, :])
        nc.sync.dma_start(out=prot_sb, in_=prot[:, :])

        def _hs_kick(eng, dc, lo, hi):
            m = _dcm(dc)
            eng.dma_start(out=hs_big[:m, dc, lo:hi], in_=hsT[dc * 128 : dc * 128 + m, lo:hi])

        def _wqkv_kick(eng, dc):
            m = _dcm(dc)
            eng.dma_start(out=wqkv_sb[:m, dc, :], in_=wqkvT[dc * 128 : dc * 128 + m, :])

        # prefix-critical (wqkv + first-half hs, ~2.3MB) balanced across the
        # three queues; the sync queue stays light so the mid-kernel ktd row
        # duplications are not stuck behind bulk transfers.
        _hs_kick(nc.gpsimd, 4, 0, SQ)          # small 64-row chunk first
        for dc in range(3):
            _wqkv_kick(nc.gpsimd, dc)
        _hs_kick(nc.sync, 0, 0, SQ)
        _hs_kick(nc.sync, 1, 0, SQ)
        _wqkv_kick(nc.sync, 3)
        _wqkv_kick(nc.sync, 4)
        for r in range(1, 4):
            nc.scalar.dma_start(
                out=trig_sb[r * 32 : (r + 1) * 32, :], in_=trig_sb[0:32, :]
            )
        _hs_kick(nc.scalar, 2, 0, SQ)
        _hs_kick(nc.scalar, 3, 0, SQ)
        # second-half hs + wo follow on the scalar queue (needed later)
        for dc in range(NDC):
            _hs_kick(nc.scalar, dc, SQ, S)
        nc.scalar.dma_start(
            out=wo_sb[:, 0:4, :], in_=woT[0:512, :].rearrange("(n p) c -> p n c", p=128)
        )
        nc.scalar.dma_start(out=wo_sb[:64, 4, :], in_=woT[512:576, :])

        cos_ap = trig_sb[:, 0:S]
        sin_ap = trig_sb[:, S : 2 * S]

        # persistent activation tensors
        qt_sb = [qtp.tile([128, SQ], F16, tag=f"qt{c}", name=f"qt{c}") for c in range(NEC)]
        ktd_sb = [ktp.tile([128, S], F16, tag=f"ktd{g}", name=f"ktd{g}") for g in range(G)]
        va_sb = [vap.tile([128, 3 * 65], F16, tag=f"va{kc}", name=f"va{kc}") for kc in range(NKC)]
        ot_sb = [otp.tile([128, SQ], F16, tag=f"ot{c}", name=f"ot{c}") for c in range(NEC)]

        # ---------------- rope (split into a/b halves for pipelining) ----
        def rope_a(src_ps, n, so):
            """Evacuate PSUM proj -> fp16 raw, and compute raw*cos."""
            raw = ropep.tile([128, SQ], F16, tag="rraw", name="raw")
            nc.vector.tensor_copy(raw[:n, :], src_ps[:n, :])
            tcm = ropep.tile([128, SQ], F16, tag="rtcm", name="tcm")
            nc.vector.tensor_mul(tcm[:n, :], raw[:n, :], cos_ap[:n, so : so + SQ])
            return raw, tcm

        def rope_b(raw, tcm, n, so, dst_writes):
            """rot = P.T @ raw (PE); dst = tcm + rot*sin."""
            rot = ps.tile([128, SQ], F32, tag="pj", name="rot")
            for sb in range(2):
                nc.tensor.matmul(
                    rot[:n, sb * 512 : (sb + 1) * 512],
                    lhsT=prot_sb[:n, :n],
                    rhs=raw[:n, sb * 512 : (sb + 1) * 512],
                    start=True,
                    stop=True,
                )
            tsm = ropep.tile([128, SQ], F16, tag="rtsm", name="tsm")
            nc.vector.tensor_mul(tsm[:n, :], rot[:n, :], sin_ap[:n, so : so + SQ])
            for dst, row, r in dst_writes:
                nc.vector.tensor_add(dst, tcm[row : row + r, :], tsm[row : row + r, :])

        # ---------------- projections ----------------
        def k_proj_mm(piece, ch, part):
            # part: 0 -> dc 0-2, 1 -> dc 3-4, None -> all (dc-major so the
            # stationary weight load is shared between the two seq halves)
            roff, m = (0, 128) if ch == 0 else (128, 64)
            so = piece * SQ
            if part != 1:
                kps = ps.tile([128, SQ], F32, tag="pj", name=f"kps{piece}{ch}")
                k_proj_mm.cur = kps
            else:
                kps = k_proj_mm.cur
            dcs = {0: range(0, 3), 1: range(3, NDC), None: range(NDC)}[part]
            for dc in dcs:
                p = _dcm(dc)
                for s2 in range(2):
                    nc.tensor.matmul(
                        kps[:m, s2 * 512 : (s2 + 1) * 512],
                        lhsT=wqkv_sb[:p, dc, 576 + roff : 576 + roff + m],
                        rhs=hs_big[:p, dc, so + s2 * 512 : so + (s2 + 1) * 512],
                        start=(dc == 0),
                        stop=(dc == NDC - 1),
                    )
            return kps

        def k_rope(piece, ch, kps):
            m = 128 if ch == 0 else 64
            so = piece * SQ
            raw, tcm = rope_a(kps, m, so)
            if ch == 0:
                writes = [
                    (ktd_sb[0][0:64, so : so + SQ], 0, 64),
                    (ktd_sb[1][0:64, so : so + SQ], 64, 64),
                ]
                groups = (0, 1)
            else:
                writes = [(ktd_sb[2][0:64, so : so + SQ], 0, 64)]
                groups = (2,)
            rope_b(raw, tcm, m, so, writes)
            for g in groups:
                nc.sync.dma_start(
                    out=ktd_sb[g][64:128, so : so + SQ],
                    in_=ktd_sb[g][0:64, so : so + SQ],
                )

        def q_proj_mm(c, part):
            m = min(128, H - c * 128)
            if part != 1:
                qps = ps.tile([128, SQ], F32, tag="pj", name=f"qps{c}")
                q_proj_mm.cur = qps
            else:
                qps = q_proj_mm.cur
            dcs = {0: range(0, 3), 1: range(3, NDC), None: range(NDC)}[part]
            for dc in dcs:
                p = _dcm(dc)
                for s2 in range(2):
                    nc.tensor.matmul(
                        qps[:m, s2 * 512 : (s2 + 1) * 512],
                        lhsT=wqkv_sb[:p, dc, c * 128 : c * 128 + m],
                        rhs=hs_big[:p, dc, s2 * 512 : (s2 + 1) * 512],
                        start=(dc == 0),
                        stop=(dc == NDC - 1),
                    )
            return qps

        def q_rope(c, qps):
            m = min(128, H - c * 128)
            raw, tcm = rope_a(qps, m, 0)
            rope_b(raw, tcm, m, 0, [(qt_sb[c][0:m, :], 0, m)])
            if c == 4:
                # duplicate head-8 rows into upper partitions for the row-tiled
                # qb=1 score matmul of the final (single-head) pair
                nc.sync.dma_start(
                    out=qt_sb[4][64:128, 512:1024], in_=qt_sb[4][0:64, 512:1024]
                )

        def v_proj(kc):
            vps = ps.tile([128, SQ], F32, tag="pj", name=f"vps{kc}")
            for dc in range(NDC):
                p = _dcm(dc)
                nc.tensor.matmul(
                    vps[:, :KV],
                    lhsT=hs_big[:p, dc, kc * 128 : (kc + 1) * 128],
                    rhs=wqkv_sb[:p, dc, 768:960],
                    start=(dc == 0),
                    stop=(dc == NDC - 1),
                )
            # only the 3 "ones" columns need the memset; V overwrites the rest
            nc.vector.memset(
                va_sb[kc].rearrange("p (g w) -> p g w", g=G)[:, :, 64:65], 1.0
            )
            dst = va_sb[kc].rearrange("p (g w) -> p g w", g=G)[:, :, 0:64]
            srcv = vps[:, :KV].rearrange("p (g w) -> p g w", g=G)
            nc.vector.tensor_copy(dst, srcv)

        # ---------------- PE warm-up ----------------
        # The HAM clock gate holds the PE at 1.2 GHz until it has been busy
        # ~3.4us.  Issue dependency-free dummy matmuls on scratch SBUF so the
        # array is at 2.4 GHz by the time the first real projection data
        # lands (~12-14us in, DMA-bound).
        wu_s = wpool.tile([128, 512], F16, tag="wus", name="wu_s")
        nc.vector.memset(wu_s, 0.125)
        wu_ps = ps.tile([128, SQ], F32, tag="pj", name="wu_ps")
        for _ in range(40):
            nc.tensor.matmul(
                wu_ps[:, 0:512], lhsT=wu_s[:, 0:128], rhs=wu_s, start=True, stop=True
            )

        # ---------------- prefix: work before attention ----------
        # K both pieces (groups 0/1), Q chunk 0, V chunks 0-1.  Runs warm
        # thanks to the dummies above, overlapping the remaining DMA.
        kps = k_proj_mm(0, 0, None)
        k_rope(0, 0, kps)
        qps = q_proj_mm(0, None)
        q_rope(0, qps)
        for kc in range(4):
            v_proj(kc)

        # ---------------- extras: deferred projections -------------------
        extras = deque()

        def add_proj_extras(fn_mm, fn_rope, *args, spread=0):
            st = {}

            def e1():
                st["ps"] = fn_mm(*args, 0)

            def e2():
                fn_mm(*args, 1)

            def e3():
                fn_rope(*args, st["ps"])

            for e in (e1, e2, e3):
                extras.append(e)
                for _ in range(spread):
                    extras.append(None)

        add_proj_extras(k_proj_mm, k_rope, 1, 0)  # K piece 1 (needed at kc=8)
        for kc in range(4, NKC):
            extras.append(lambda kc=kc: v_proj(kc))
        add_proj_extras(q_proj_mm, q_rope, 1, spread=1)
        add_proj_extras(q_proj_mm, q_rope, 2, spread=1)
        add_proj_extras(q_proj_mm, q_rope, 3, spread=1)
        add_proj_extras(k_proj_mm, k_rope, 0, 1, spread=1)
        add_proj_extras(k_proj_mm, k_rope, 1, 1, spread=1)
        add_proj_extras(q_proj_mm, q_rope, 4, spread=1)

        # ---------------- attention ----------------
        def attn_pair(pi):
            pair = PAIRS[pi]
            two = len(pair) == 2
            hA = pair[0]
            gA = hA // 3
            gB = pair[1] // 3 if two else gA
            c = hA // 2
            for qb in (0, 1) if two else (0,):
                avA = ps.tile([65, 512], F32, tag="avA", bufs=1, name="avA")
                avB = ps.tile([65, 512], F32, tag="avB", bufs=1, name="avB")
                at_tiles = [None] * NKC

                def emit_av(kcav):
                    nc.tensor.matmul(
                        avA,
                        lhsT=va_sb[kcav][:, gA * 65 : gA * 65 + 65],
                        rhs=at_tiles[kcav][:, 0:512],
                        start=(kcav == 0),
                        stop=(kcav == NKC - 1),
                    )
                    nc.tensor.matmul(
                        avB,
                        lhsT=va_sb[kcav][:, gB * 65 : gB * 65 + 65],
                        rhs=at_tiles[kcav][:, 512:1024],
                        start=(kcav == 0),
                        stop=(kcav == NKC - 1),
                    )

                for kc in range(NKC):
                    st = ps.tile([128, 1024], F32, tag="st", bufs=2, name="st")
                    if two:
                        rhsA = qt_sb[c][0:64, qb * 512 : (qb + 1) * 512]
                        rhsB = qt_sb[c][64:128, qb * 512 : (qb + 1) * 512]
                    else:
                        rhsA = qt_sb[c][0:64, 0:512]
                        rhsB = qt_sb[c][64:128, 512:1024]
                    nc.tensor.matmul(
                        st[:, 0:512],
                        lhsT=ktd_sb[gA][0:64, kc * 128 : (kc + 1) * 128],
                        rhs=rhsA,
                        start=True,
                        stop=True,
                    )
                    nc.tensor.matmul(
                        st[:, 512:1024],
                        lhsT=ktd_sb[gB][64:128, kc * 128 : (kc + 1) * 128],
                        rhs=rhsB,
                        start=True,
                        stop=True,
                    )
                    at_t = attnp.tile([128, 1024], F16, tag="at", name="at")
                    nc.scalar.activation(
                        at_t, st, mybir.ActivationFunctionType.Exp, scale=SCALE
                    )
                    at_tiles[kc] = at_t
                    if extras:
                        e = extras.popleft()
                        if e is not None:
                            e()
                    if kc > 0:
                        emit_av(kc - 1)

                emit_av(NKC - 1)

                targets = (
                    [(hA, avA, qb), (pair[1], avB, qb)]
                    if two
                    else [(hA, avA, 0), (hA, avB, 1)]
                )
                # Evacuate both av PSUM banks first (so the next pair's AV
                # matmuls can reuse them ASAP), then do the arithmetic on the
                # SBUF copies.  The denominator row must be staged to a
                # partition-0 tile (custom DVE ops drop partition offsets).
                stage = []
                for h, av, qbx in targets:
                    avs = miscp.tile([64, 512], F32, tag="avs", name="avs")
                    nc.vector.tensor_copy(avs, av[0:64, :])
                    dn = miscp.tile([1, 512], F32, tag="dn", name="dn")
                    nc.vector.tensor_copy(dn, av[64:65, :])
                    stage.append((h, qbx, avs, dn))
                rds = []
                for h, qbx, avs, dn in stage:
                    rd = miscp.tile([1, 512], F32, tag="rd", name="rd")
                    nc.vector.reciprocal_approx_fast(out=rd, in_=dn)
                    rds.append(rd)
                for (h, qbx, avs, dn), rd in zip(stage, rds):
                    bc = miscp.tile([64, 512], F32, tag="bc", name="bc")
                    nc.gpsimd.partition_broadcast(bc, rd)
                    row = (h % 2) * 64
                    # final scale on gpsimd (SBUF-only operands) to keep the
                    # DVE free for the offloaded exp work
                    nc.gpsimd.tensor_mul(
                        ot_sb[h // 2][row : row + 64, qbx * 512 : (qbx + 1) * 512],
                        avs,
                        bc,
                    )

        for pi in range(len(PAIRS)):
            attn_pair(pi)

        # ---------------- output projection ----------------
        # Keep the PE busy through the final normalize (DVE/gpsimd) so HAM
        # does not re-throttle right before the o_proj matmuls.
        wu2 = ps.tile([128, SQ], F32, tag="pj", name="wu2")
        for _ in range(10):
            nc.tensor.matmul(
                wu2[:, 0:512], lhsT=wu_s[:, 0:128], rhs=wu_s, start=True, stop=True
            )
        # attention is done: the "st" PSUM buffers (2x) are free, giving a
        # double-buffered ec pipeline; fp16 output halves the writeback.
        dma_engs = [nc.sync, nc.gpsimd, nc.scalar]
        for ec in range(NEC):
            m = min(128, H - ec * 128)
            ft = ps.tile([128, SQ], F32, tag="st", bufs=2, name=f"ft{ec}")
            for sb in range(2):
                for cc in range(NEC):
                    k = _dcm(cc)
                    nc.tensor.matmul(
                        ft[:m, sb * 512 : (sb + 1) * 512],
                        lhsT=wo_sb[:k, cc, ec * 128 : ec * 128 + m],
                        rhs=ot_sb[cc][:k, sb * 512 : (sb + 1) * 512],
                        start=(cc == 0),
                        stop=(cc == NEC - 1),
                    )
                fts = miscp.tile([128, 512], F16, tag="fts", name="fts")
                if sb == 0:
                    nc.scalar.copy(fts[:m, :], ft[:m, 0:512])
                else:
                    nc.vector.tensor_copy(fts[:m, :], ft[:m, 512:1024])
                dma_engs[(ec * 2 + sb) % 3].dma_start(
                    out=out[ec * 128 : ec * 128 + m, sb * 512 : (sb + 1) * 512],
                    in_=fts[:m, :],
                )


_NC_CACHE = {}


def _get_nc():
    if "nc" not in _NC_CACHE:
        _NC_CACHE["nc"] = _build_bass()
    return _NC_CACHE["nc"]


def kernel(hidden_states, wq, wk, wv, wo):
    cos32, sin32, prot = _host_tables()

    wqkv = np.empty((H, 960), np.float16)
    wqkv[:, 0:H] = wq.T.astype(np.float16)
    wqkv[:, H : H + KV] = wk.T.astype(np.float16)
    wqkv[:, H + KV : H + 2 * KV] = wv.T.astype(np.float16)
    wo16 = wo.T.astype(np.float16)

    trig0 = np.concatenate([cos32, sin32], axis=1)
    trig1 = np.concatenate(
        [np.roll(cos32, -SQ, axis=1), np.roll(sin32, -SQ, axis=1)], axis=1
    )

    in_maps = []
    core_ids = list(range(8))
    for c in core_ids:
        b, half = c // 2, c % 2
        hsT16 = hidden_states[b].T.astype(np.float16)
        if half == 1:
            # roll so this core's queries sit at columns [0, SQ); keys keep
            # their correct rope position via the equally-rolled cos/sin.
            hsT16 = np.roll(hsT16, -SQ, axis=1)
        in_maps.append(
            {
                "hsT": hsT16,
                "wqkvT": wqkv,
                "woT": wo16,
                "trig": trig0 if half == 0 else trig1,
                "prot": prot,
            }
        )

    global _LAST_IN_MAPS
    _LAST_IN_MAPS = in_maps
    nc = _get_nc()
    res = run_bass_kernel_spmd(nc, in_maps, core_ids=core_ids)

    out = np.empty((B, S, H), np.float32)
    for c in core_ids:
        b, half = c // 2, c % 2
        out[b, half * SQ : (half + 1) * SQ, :] = res.results[c]["o"].T.astype(
            np.float32
        )
    return out


if __name__ == "__main__":
    rng = np.random.default_rng(0)
    hs = rng.standard_normal((B, S, H), dtype=np.float32)
    s = 1.0 / np.sqrt(H)
    wq = rng.standard_normal((H, H), dtype=np.float32) * s
    wk = rng.standard_normal((KV, H), dtype=np.float32) * s
    wv = rng.standard_normal((KV, H), dtype=np.float32) * s
    wo = rng.standard_normal((H, H), dtype=np.float32) * s
    o = kernel(hidden_states=hs, wq=wq, wk=wk, wv=wv, wo=wo)
    print(o.shape, o.dtype, np.abs(o).mean())


# revision 35
# speedup vs baseline: 1.4623x; 1.4324x over previous
"""Trainium2 Bass kernel for GQA attention (B=4, S=2048, H=576, 9 heads / 3 KV groups, RoPE).

Sharding: 8 cores = (batch b, seq-half) pairs. Each core computes the full
attention output for 1024 query rows of one batch element (keys/values over
the full 2048 positions of that batch element are recomputed locally).

Schedule: attention (the ACT-engine exp stream, which is the bottleneck)
starts as soon as K groups 0/1 (first seq half) + Q chunk 0 are projected;
all remaining projections (V chunks, Q chunks 1-4, K second half / group 2)
are drip-fed into the attention pair loop as small "extras" that fill
TensorE slack while ACT runs.

RoPE is computed as q' = q*cos + (P.T @ q)*sin where P is a constant 128x128
block permutation carrying the rotate-half signs (one small matmul replaces
the per-32-row DVE cross-multiplies).

Layout: features on partitions, seq on free dim (all "transposed"):
  QT = wq @ hsT, KT = wk @ hsT, V natural [s, hv]
  ST[k, q] = K-chunk.T @ QT -> exp on ACT -> attnT fp16
  av[hd+1, q] = [V | ones].T @ attnT  (ones row = softmax denominator)
  outT = woT.T @ (av[0:64] / av[64])
Head pairs (2h, 2h+1) run concurrently on the PE array via 64x128 row tiling
(lhsT/rhs in partitions 0-63 vs 64-127). Matmul inputs fp16, fp32 PSUM.
"""

import sys

if "/opt/trn_rl_repo" not in sys.path:
    sys.path.insert(0, "/opt/trn_rl_repo")

import numpy as np

import concourse.bass as bass
import concourse.mybir as mybir
import concourse.tile as tile
from concourse import bacc
from concourse.bass_utils import run_bass_kernel_spmd

F16 = mybir.dt.float16
F32 = mybir.dt.float32

B = 4
S = 2048
SQ = 1024  # query rows per core
H = 576
NH = 9
HD = 64
KV = 192
G = 3
ROPE_THETA = 10000.0
SCALE = 1.0 / 8.0  # 1/sqrt(HD)
# Schraudolph exp2-bit-trick constants: exp(SCALE*x) = 2^(SCALE*log2(e)*x)
EXP2_A1 = SCALE * 1.4426950408889634 * 1024.0
EXP2_A2 = 15360.0 - 44.0

NDC = 5  # contraction chunks over hidden (4*128 + 64)
NEC = 5  # output feature chunks (4*128 + 64)
NKC = S // 128  # 16 key chunks
PAIRS = [(0, 1), (2, 3), (4, 5), (6, 7), (8,)]


def _dcm(dc):
    return min(128, H - dc * 128)  # 128,128,128,128,64


def _host_tables():
    inv_freq = 1.0 / (ROPE_THETA ** (np.arange(0, HD, 2, dtype=np.float32) / HD))
    t = np.arange(S, dtype=np.float32)
    freqs = np.einsum("i,j->ij", inv_freq, t)  # [32, S]
    cos32 = np.cos(freqs).astype(np.float16)
    sin32 = np.sin(freqs).astype(np.float16)
    # P[c, r]: rot = P.T @ x; rot[r] = -x[r+32] (r in [0,32)), +x[r-32] (r in [32,64))
    prot = np.zeros((128, 128), np.float16)
    for b0 in (0, 64):
        for j in range(32):
            prot[b0 + j + 32, b0 + j] = -1.0
            prot[b0 + j, b0 + j + 32] = 1.0
    return cos32, sin32, prot


def _build_bass():
    nc = bacc.Bacc("TRN2", target_bir_lowering=False)

    hsT = nc.declare_dram_parameter("hsT", [H, S], F16, isOutput=False)
    wqkvT = nc.declare_dram_parameter("wqkvT", [H, 960], F16, isOutput=False)
    woT = nc.declare_dram_parameter("woT", [H, H], F16, isOutput=False)
    trig = nc.declare_dram_parameter("trig", [32, 2 * S], F16, isOutput=False)
    prot = nc.declare_dram_parameter("prot", [128, 128], F16, isOutput=False)
    out = nc.declare_dram_parameter("o", [H, SQ], F16, isOutput=True)

    with tile.TileContext(nc) as tc:
        kernel_body(nc, tc, hsT, wqkvT, woT, trig, prot, out)

    nc.compile()
    return nc


def kernel_body(nc, tc, hsT, wqkvT, woT, trig, prot, out):
    import contextlib
    from collections import deque

    ctx = contextlib.ExitStack()
    with ctx:
        wpool = ctx.enter_context(tc.tile_pool(name="w", bufs=1))
        qtp = ctx.enter_context(tc.tile_pool(name="qt", bufs=1))
        ktp = ctx.enter_context(tc.tile_pool(name="kt", bufs=1))
        vap = ctx.enter_context(tc.tile_pool(name="va", bufs=1))
        otp = ctx.enter_context(tc.tile_pool(name="ot", bufs=1))
        ropep = ctx.enter_context(tc.tile_pool(name="rope", bufs=2))
        attnp = ctx.enter_context(tc.tile_pool(name="attn", bufs=6))
        miscp = ctx.enter_context(tc.tile_pool(name="misc", bufs=3))
        ps = ctx.enter_context(tc.tile_pool(name="ps", bufs=1, space="PSUM"))

        # ---------------- inputs to SBUF ----------------
        # sync queue: only small latency-critical transfers (trig, prot, the
        # ktd/qt row-duplications later, output).  Bulk loads go on the
        # gpsimd and scalar queues, split per (dc, seq-half) so the first
        # K/Q projection matmuls can start as soon as their chunk lands.
        hs_big = wpool.tile([128, NDC, S], F16, tag="hsb", name="hs_big")
        wqkv_sb = wpool.tile([128, NDC, 960], F16, tag="wqkv", name="wqkv_sb")
        wo_sb = wpool.tile([128, NEC, H], F16, tag="wo", name="wo_sb")
        trig_sb = wpool.tile([128, 2 * S], F16, tag="trig", name="trig_sb")
        prot_sb = wpool.tile([128, 128], F16, tag="prot", name="prot_sb")

        # sync (Q1): trig/prot then first-half hs chunks 0-2, later dups.
        # gpsimd (Q0): all wqkv chunks.  scalar (Q10): trig replication,
        # first-half hs chunks 3-4, then second-half hs, then wo.
        nc.sync.dma_start(out=trig_sb[0:32, :], in_=trig[:, :])
        nc.sync.dma_start(out=prot_sb, in_=prot[:, :])

        def _hs_kick(eng, dc, lo, hi):
            m = _dcm(dc)
            eng.dma_start(out=hs_big[:m, dc, lo:hi], in_=hsT[dc * 128 : dc * 128 + m, lo:hi])

        def _wqkv_kick(eng, dc):
            m = _dcm(dc)
            eng.dma_start(out=wqkv_sb[:m, dc, :], in_=wqkvT[dc * 128 : dc * 128 + m, :])

        # prefix-critical (wqkv + first-half hs, ~2.3MB) balanced across the
        # three queues; the sync queue stays light so the mid-kernel ktd row
        # duplications are not stuck behind bulk transfers.
        _hs_kick(nc.gpsimd, 4, 0, SQ)          # small 64-row chunk first
        for dc in range(3):
            _wqkv_kick(nc.gpsimd, dc)
        _hs_kick(nc.sync, 0, 0, SQ)
        _hs_kick(nc.sync, 1, 0, SQ)
        _wqkv_kick(nc.sync, 3)
        _wqkv_kick(nc.sync, 4)
        for r in range(1, 4):
            nc.scalar.dma_start(
                out=trig_sb[r * 32 : (r + 1) * 32, :], in_=trig_sb[0:32, :]
            )
        _hs_kick(nc.scalar, 2, 0, SQ)
        _hs_kick(nc.scalar, 3, 0, SQ)
        # second-half hs + wo follow on the scalar queue (needed later)
        for dc in range(NDC):
            _hs_kick(nc.scalar, dc, SQ, S)
        nc.scalar.dma_start(
            out=wo_sb[:, 0:4, :], in_=woT[0:512, :].rearrange("(n p) c -> p n c", p=128)
        )
        nc.scalar.dma_start(out=wo_sb[:64, 4, :], in_=woT[512:576, :])

        cos_ap = trig_sb[:, 0:S]
        sin_ap = trig_sb[:, S : 2 * S]

        # persistent activation tensors
        qt_sb = [qtp.tile([128, SQ], F16, tag=f"qt{c}", name=f"qt{c}") for c in range(NEC)]
        ktd_sb = [ktp.tile([128, S], F16, tag=f"ktd{g}", name=f"ktd{g}") for g in range(G)]
        va_sb = [vap.tile([128, 3 * 65], F16, tag=f"va{kc}", name=f"va{kc}") for kc in range(NKC)]
        ot_sb = [otp.tile([128, SQ], F16, tag=f"ot{c}", name=f"ot{c}") for c in range(NEC)]

        # ---------------- rope (split into a/b halves for pipelining) ----
        def rope_a(src_ps, n, so):
            """Evacuate PSUM proj -> fp16 raw, and compute raw*cos."""
            raw = ropep.tile([128, SQ], F16, tag="rraw", name="raw")
            nc.vector.tensor_copy(raw[:n, :], src_ps[:n, :])
            tcm = ropep.tile([128, SQ], F16, tag="rtcm", name="tcm")
            nc.vector.tensor_mul(tcm[:n, :], raw[:n, :], cos_ap[:n, so : so + SQ])
            return raw, tcm

        def rope_b(raw, tcm, n, so, dst_writes):
            """rot = P.T @ raw (PE); dst = tcm + rot*sin."""
            rot = ps.tile([128, SQ], F32, tag="pj", name="rot")
            for sb in range(2):
                nc.tensor.matmul(
                    rot[:n, sb * 512 : (sb + 1) * 512],
                    lhsT=prot_sb[:n, :n],
                    rhs=raw[:n, sb * 512 : (sb + 1) * 512],
                    start=True,
                    stop=True,
                )
            tsm = ropep.tile([128, SQ], F16, tag="rtsm", name="tsm")
            nc.vector.tensor_mul(tsm[:n, :], rot[:n, :], sin_ap[:n, so : so + SQ])
            for dst, row, r in dst_writes:
                nc.vector.tensor_add(dst, tcm[row : row + r, :], tsm[row : row + r, :])

        # ---------------- projections ----------------
        def k_proj_mm(piece, ch, part):
            # part: 0 -> dc 0-2, 1 -> dc 3-4, None -> all (dc-major so the
            # stationary weight load is shared between the two seq halves)
            roff, m = (0, 128) if ch == 0 else (128, 64)
            so = piece * SQ
            if part != 1:
                kps = ps.tile([128, SQ], F32, tag="pj", name=f"kps{piece}{ch}")
                k_proj_mm.cur = kps
            else:
                kps = k_proj_mm.cur
            dcs = {0: range(0, 3), 1: range(3, NDC), None: range(NDC)}[part]
            for dc in dcs:
                p = _dcm(dc)
                for s2 in range(2):
                    nc.tensor.matmul(
                        kps[:m, s2 * 512 : (s2 + 1) * 512],
                        lhsT=wqkv_sb[:p, dc, 576 + roff : 576 + roff + m],
                        rhs=hs_big[:p, dc, so + s2 * 512 : so + (s2 + 1) * 512],
                        start=(dc == 0),
                        stop=(dc == NDC - 1),
                    )
            return kps

        def k_rope(piece, ch, kps):
            m = 128 if ch == 0 else 64
            so = piece * SQ
            raw, tcm = rope_a(kps, m, so)
            if ch == 0:
                writes = [
                    (ktd_sb[0][0:64, so : so + SQ], 0, 64),
                    (ktd_sb[1][0:64, so : so + SQ], 64, 64),
                ]
                groups = (0, 1)
            else:
                writes = [(ktd_sb[2][0:64, so : so + SQ], 0, 64)]
                groups = (2,)
            rope_b(raw, tcm, m, so, writes)
            for g in groups:
                nc.sync.dma_start(
                    out=ktd_sb[g][64:128, so : so + SQ],
                    in_=ktd_sb[g][0:64, so : so + SQ],
                )

        def q_proj_mm(c, part):
            m = min(128, H - c * 128)
            if part != 1:
                qps = ps.tile([128, SQ], F32, tag="pj", name=f"qps{c}")
                q_proj_mm.cur = qps
            else:
                qps = q_proj_mm.cur
            dcs = {0: range(0, 3), 1: range(3, NDC), None: range(NDC)}[part]
            for dc in dcs:
                p = _dcm(dc)
                for s2 in range(2):
                    nc.tensor.matmul(
                        qps[:m, s2 * 512 : (s2 + 1) * 512],
                        lhsT=wqkv_sb[:p, dc, c * 128 : c * 128 + m],
                        rhs=hs_big[:p, dc, s2 * 512 : (s2 + 1) * 512],
                        start=(dc == 0),
                        stop=(dc == NDC - 1),
                    )
            return qps

        def q_rope(c, qps):
            m = min(128, H - c * 128)
            raw, tcm = rope_a(qps, m, 0)
            rope_b(raw, tcm, m, 0, [(qt_sb[c][0:m, :], 0, m)])
            if c == 4:
                # duplicate head-8 rows into upper partitions for the row-tiled
                # qb=1 score matmul of the final (single-head) pair
                nc.sync.dma_start(
                    out=qt_sb[4][64:128, 512:1024], in_=qt_sb[4][0:64, 512:1024]
                )

        def v_proj(kc):
            vps = ps.tile([128, SQ], F32, tag="pj", name=f"vps{kc}")
            for dc in range(NDC):
                p = _dcm(dc)
                nc.tensor.matmul(
                    vps[:, :KV],
                    lhsT=hs_big[:p, dc, kc * 128 : (kc + 1) * 128],
                    rhs=wqkv_sb[:p, dc, 768:960],
                    start=(dc == 0),
                    stop=(dc == NDC - 1),
                )
            # only the 3 "ones" columns need the memset; V overwrites the rest
            nc.vector.memset(
                va_sb[kc].rearrange("p (g w) -> p g w", g=G)[:, :, 64:65], 1.0
            )
            dst = va_sb[kc].rearrange("p (g w) -> p g w", g=G)[:, :, 0:64]
            srcv = vps[:, :KV].rearrange("p (g w) -> p g w", g=G)
            nc.vector.tensor_copy(dst, srcv)

        # ---------------- PE warm-up ----------------
        # The HAM clock gate holds the PE at 1.2 GHz until it has been busy
        # ~3.4us.  Issue dependency-free dummy matmuls on scratch SBUF so the
        # array is at 2.4 GHz by the time the first real projection data
        # lands (~12-14us in, DMA-bound).
        wu_s = wpool.tile([128, 512], F16, tag="wus", name="wu_s")
        nc.vector.memset(wu_s, 0.125)
        wu_ps = ps.tile([128, SQ], F32, tag="pj", name="wu_ps")
        for _ in range(40):
            nc.tensor.matmul(
                wu_ps[:, 0:512], lhsT=wu_s[:, 0:128], rhs=wu_s, start=True, stop=True
            )

        # ---------------- prefix: work before attention ----------
        # K both pieces (groups 0/1), Q chunk 0, V chunks 0-1.  Runs warm
        # thanks to the dummies above, overlapping the remaining DMA.
        kps = k_proj_mm(0, 0, None)
        k_rope(0, 0, kps)
        qps = q_proj_mm(0, None)
        q_rope(0, qps)
        for kc in range(4):
            v_proj(kc)

        # ---------------- extras: deferred projections -------------------
        extras = deque()

        def add_proj_extras(fn_mm, fn_rope, *args, spread=0):
            st = {}

            def e1():
                st["ps"] = fn_mm(*args, 0)

            def e2():
                fn_mm(*args, 1)

            def e3():
                fn_rope(*args, st["ps"])

            for e in (e1, e2, e3):
                extras.append(e)
                for _ in range(spread):
                    extras.append(None)

        add_proj_extras(k_proj_mm, k_rope, 1, 0)  # K piece 1 (needed at kc=8)
        for kc in range(4, NKC):
            extras.append(lambda kc=kc: v_proj(kc))
        add_proj_extras(q_proj_mm, q_rope, 1, spread=2)
        add_proj_extras(q_proj_mm, q_rope, 2, spread=3)
        add_proj_extras(q_proj_mm, q_rope, 3, spread=3)
        add_proj_extras(k_proj_mm, k_rope, 0, 1, spread=3)
        add_proj_extras(k_proj_mm, k_rope, 1, 1, spread=3)
        add_proj_extras(q_proj_mm, q_rope, 4, spread=3)

        # ---------------- attention ----------------
        def attn_pair(pi):
            pair = PAIRS[pi]
            two = len(pair) == 2
            hA = pair[0]
            gA = hA // 3
            gB = pair[1] // 3 if two else gA
            c = hA // 2
            for qb in (0, 1) if two else (0,):
                avA = ps.tile([65, 512], F32, tag="avA", bufs=1, name="avA")
                avB = ps.tile([65, 512], F32, tag="avB", bufs=1, name="avB")
                at_tiles = [None] * NKC

                def emit_av(kcav):
                    nc.tensor.matmul(
                        avA,
                        lhsT=va_sb[kcav][:, gA * 65 : gA * 65 + 65],
                        rhs=at_tiles[kcav][:, 0:512],
                        start=(kcav == 0),
                        stop=(kcav == NKC - 1),
                    )
                    nc.tensor.matmul(
                        avB,
                        lhsT=va_sb[kcav][:, gB * 65 : gB * 65 + 65],
                        rhs=at_tiles[kcav][:, 512:1024],
                        start=(kcav == 0),
                        stop=(kcav == NKC - 1),
                    )

                for kc in range(NKC):
                    st = ps.tile([128, 1024], F32, tag="st", bufs=2, name="st")
                    if two:
                        rhsA = qt_sb[c][0:64, qb * 512 : (qb + 1) * 512]
                        rhsB = qt_sb[c][64:128, qb * 512 : (qb + 1) * 512]
                    else:
                        rhsA = qt_sb[c][0:64, 0:512]
                        rhsB = qt_sb[c][64:128, 512:1024]
                    nc.tensor.matmul(
                        st[:, 0:512],
                        lhsT=ktd_sb[gA][0:64, kc * 128 : (kc + 1) * 128],
                        rhs=rhsA,
                        start=True,
                        stop=True,
                    )
                    nc.tensor.matmul(
                        st[:, 512:1024],
                        lhsT=ktd_sb[gB][64:128, kc * 128 : (kc + 1) * 128],
                        rhs=rhsB,
                        start=True,
                        stop=True,
                    )
                    at_t = attnp.tile([128, 1024], F16, tag="at", name="at")
                    nc.scalar.activation(
                        at_t, st, mybir.ActivationFunctionType.Exp, scale=SCALE
                    )
                    at_tiles[kc] = at_t
                    if extras:
                        e = extras.popleft()
                        if e is not None:
                            e()
                    if kc > 0:
                        emit_av(kc - 1)

                emit_av(NKC - 1)

                targets = (
                    [(hA, avA, qb), (pair[1], avB, qb)]
                    if two
                    else [(hA, avA, 0), (hA, avB, 1)]
                )
                # Evacuate both av PSUM banks first (so the next pair's AV
                # matmuls can reuse them ASAP), then do the arithmetic on the
                # SBUF copies.  The denominator row must be staged to a
                # partition-0 tile (custom DVE ops drop partition offsets).
                stage = []
                for h, av, qbx in targets:
                    avs = miscp.tile([64, 512], F32, tag="avs", name="avs")
                    nc.vector.tensor_copy(avs, av[0:64, :])
                    dn = miscp.tile([1, 512], F32, tag="dn", name="dn")
                    nc.vector.tensor_copy(dn, av[64:65, :])
                    stage.append((h, qbx, avs, dn))
                rds = []
                for h, qbx, avs, dn in stage:
                    rd = miscp.tile([1, 512], F32, tag="rd", name="rd")
                    nc.vector.reciprocal_approx_fast(out=rd, in_=dn)
                    rds.append(rd)
                for (h, qbx, avs, dn), rd in zip(stage, rds):
                    bc = miscp.tile([64, 512], F32, tag="bc", name="bc")
                    nc.gpsimd.partition_broadcast(bc, rd)
                    row = (h % 2) * 64
                    nc.vector.tensor_mul(
                        ot_sb[h // 2][row : row + 64, qbx * 512 : (qbx + 1) * 512],
                        avs,
                        bc,
                    )

        for pi in range(len(PAIRS)):
            attn_pair(pi)

        # ---------------- output projection ----------------
        # Keep the PE busy through the final normalize (DVE/gpsimd) so HAM
        # does not re-throttle right before the o_proj matmuls.
        wu2 = ps.tile([128, SQ], F32, tag="pj", name="wu2")
        for _ in range(10):
            nc.tensor.matmul(
                wu2[:, 0:512], lhsT=wu_s[:, 0:128], rhs=wu_s, start=True, stop=True
            )
        # attention is done: the "st" PSUM buffers (2x) are free, giving a
        # double-buffered ec pipeline; fp16 output halves the writeback.
        dma_engs = [nc.sync, nc.gpsimd, nc.scalar]
        for ec in range(NEC):
            m = min(128, H - ec * 128)
            ft = ps.tile([128, SQ], F32, tag="st", bufs=2, name=f"ft{ec}")
            for sb in range(2):
                for cc in range(NEC):
                    k = _dcm(cc)
                    nc.tensor.matmul(
                        ft[:m, sb * 512 : (sb + 1) * 512],
                        lhsT=wo_sb[:k, cc, ec * 128 : ec * 128 + m],
                        rhs=ot_sb[cc][:k, sb * 512 : (sb + 1) * 512],
                        start=(cc == 0),
                        stop=(cc == NEC - 1),
                    )
                fts = miscp.tile([128, 512], F16, tag="fts", name="fts")
                if sb == 0:
                    nc.scalar.copy(fts[:m, :], ft[:m, 0:512])
                else:
                    nc.vector.tensor_copy(fts[:m, :], ft[:m, 512:1024])
                dma_engs[(ec * 2 + sb) % 3].dma_start(
                    out=out[ec * 128 : ec * 128 + m, sb * 512 : (sb + 1) * 512],
                    in_=fts[:m, :],
                )


_NC_CACHE = {}


def _get_nc():
    if "nc" not in _NC_CACHE:
        _NC_CACHE["nc"] = _build_bass()
    return _NC_CACHE["nc"]


def kernel(hidden_states, wq, wk, wv, wo):
    cos32, sin32, prot = _host_tables()

    wqkv = np.empty((H, 960), np.float16)
    wqkv[:, 0:H] = wq.T.astype(np.float16)
    wqkv[:, H : H + KV] = wk.T.astype(np.float16)
    wqkv[:, H + KV : H + 2 * KV] = wv.T.astype(np.float16)
    wo16 = wo.T.astype(np.float16)

    trig0 = np.concatenate([cos32, sin32], axis=1)
    trig1 = np.concatenate(
        [np.roll(cos32, -SQ, axis=1), np.roll(sin32, -SQ, axis=1)], axis=1
    )

    in_maps = []
    core_ids = list(range(8))
    for c in core_ids:
        b, half = c // 2, c % 2
        hsT16 = hidden_states[b].T.astype(np.float16)
        if half == 1:
            # roll so this core's queries sit at columns [0, SQ); keys keep
            # their correct rope position via the equally-rolled cos/sin.
            hsT16 = np.roll(hsT16, -SQ, axis=1)
        in_maps.append(
            {
                "hsT": hsT16,
                "wqkvT": wqkv,
                "woT": wo16,
                "trig": trig0 if half == 0 else trig1,
                "prot": prot,
            }
        )

    global _LAST_IN_MAPS
    _LAST_IN_MAPS = in_maps
    nc = _get_nc()
    res = run_bass_kernel_spmd(nc, in_maps, core_ids=core_ids)

    out = np.empty((B, S, H), np.float32)
    for c in core_ids:
        b, half = c // 2, c % 2
        out[b, half * SQ : (half + 1) * SQ, :] = res.results[c]["o"].T.astype(
            np.float32
        )
    return out


if __name__ == "__main__":
    rng = np.random.default_rng(0)
    hs = rng.standard_normal((B, S, H), dtype=np.float32)
    s = 1.0 / np.sqrt(H)
    wq = rng.standard_normal((H, H), dtype=np.float32) * s
    wk = rng.standard_normal((KV, H), dtype=np.float32) * s
    wv = rng.standard_normal((KV, H), dtype=np.float32) * s
    wo = rng.standard_normal((H, H), dtype=np.float32) * s
    o = kernel(hidden_states=hs, wq=wq, wk=wk, wv=wv, wo=wo)
    print(o.shape, o.dtype, np.abs(o).mean())
